# revision 1
# baseline (speedup 1.0000x reference)
"""Distributed Trainium2 (Bass) kernel for nn_AnchorLoss.

Reference:
  pos  = embedding + abs_coords                     [B, N, D],  B=8, N=2048, D=2
  sq   = ||pos_i - pos_j||^2                        [B, N, N]
  loss = sum over (b,i,j) with patch_mask==1 of (1 - exp(-sq / T))

Distribution: batch b -> NeuronCore b (8 cores, data parallel). Each core
computes a partial sum; the host combines them (the all-reduce of a scalar
is free host-side since kernel() returns the full output anyway).

Math (per core):
  loss = count(mask==1) - diag(mask) - 2 * S
  S    = sum_{i<j} (msum_ij / 2) * exp(-sq_ij / T),   msum = mask + mask^T
  (exp term is symmetric in (i,j) so only the upper triangle is computed;
   diagonal terms have exp(0)=1 and cancel exactly on host.)

Kernel (per core) — the entire per-tile computation is ONE TensorEngine pass:
  The triangle is row-tiled into NTILES tiles of MT=124 rows; tile k covers
  rows [124k, 124k+m) x cols [124k, N). A K=(4+m) contraction computes
    rows 0-3:    sq via  Q[i]=[x_i,y_i,r_i,1] . K[j]=[-2x_j,-2y_j,1,r_j]
    rows 4-4+m:  an identity that accumulates a host-built penalty
                 p = -T*ln(msum/2) in {0, T*ln2, BIG}  into the same PSUM
  so PSUM holds sq + p. A ScalarE exp(-x/T) with accum_out then yields the
  weighted row-sums directly: weight exp(-p/T) is {1, 1/2, 0} (exp(-BIG/T)
  underflows to exactly 0, which also implements the triangle masking).
  Output rows m..127 are forced to BIG through dummy stationary columns
  [0,0,BIG,0] (K row 2 is all-ones) so every PSUM row is defined and exps
  to 0 — this lets several small tiles share one PSUM half and one ACTIVATE
  (the reduction is a grand sum, so mixing tiles in one accumulator is fine).
  Tiles run smallest-first (DMA size ramps with PE consumption); small tiles
  are binned 2/3/2/2 so the ScalarE queue carries 12 ACTIVATEs instead of 17
  and the first activation fires after only two small tiles.
  fp16 operands (penalties and the identity are fp16-exact; fp16 matmul runs
  the PE at full rate, unlike fp32 which is 4x slower).

  Per tile, the [K, 128] stationary block and the [K, fd] moving block are
  packed side by side in one DRAM row-block -> a single DMA per tile.
  Hand-rolled pipeline (raw bacc, no TileContext):
    sync:   DMA tile into one of NSLOT sbuf slots
    tensor: fused matmul into one of 2 PSUM halves (512-col bank chunks)
    scalar: in-place exp over the bin + accumulator read into acc[:, bin]
  Host sums the per-core [128, NBINS] accumulators in float64.
"""

from contextlib import ExitStack

import numpy as np

B, N, D = 8, 2048, 2
TEMPERATURE = 10.0
P = 128
MT = 124                      # rows per tile (K = 4 + MT <= 128)
NTILES = (N + MT - 1) // MT   # 17 (last tile has 64 rows)
CHUNK = 512                   # PSUM bank width in f32
BIG = 1536.0                  # exp(-BIG/T) == 0 in f32
LN2T = float(TEMPERATURE * np.log(2.0))  # penalty giving weight 1/2
MOVW = P + N                  # stat block (128 cols incl dummies) + moving cols
NSLOT = 6                     # mv buffers (DMA prefetch depth)
# bins preserve the descending tile order; each bin fits one 2048-col PSUM half
BINS = [[16, 15], [14, 13, 12], [11, 10], [9, 8], [7], [6], [5], [4], [3], [2], [1], [0]]

TRACE = False        # set True (see test.py) to neuron-profile the run
LAST_RESULTS = None  # BassKernelResults of the last run when TRACE

_cache = {}


def _tile_geom(k):
    i0 = k * MT
    m = min(MT, N - i0)
    fd = N - i0
    return i0, m, fd


def _build():
    from concourse import bacc, mybir

    nc = bacc.Bacc(enable_partition_id=False)
    f32 = mybir.dt.float32
    f16 = mybir.dt.float16
    mov = nc.declare_dram_parameter("mov", [NTILES * P, MOVW], f16, isOutput=False)
    out = nc.declare_dram_parameter("out", [P, len(BINS)], f32, isOutput=True)

    seq = []   # (tile_k, bin_idx, col_off)
    for bi, tks in enumerate(BINS):
        off = 0
        for k in tks:
            seq.append((k, bi, off))
            off += _tile_geom(k)[2]
        assert off <= N
    tiles_through_bin = {}
    cnt = 0
    for bi, tks in enumerate(BINS):
        cnt += len(tks)
        tiles_through_bin[bi] = cnt

    with ExitStack() as ctx:
        mvs = [
            ctx.enter_context(nc.sbuf_tensor(f"mv{j}", [P, MOVW], f16))
            for j in range(NSLOT)
        ]
        acc = ctx.enter_context(nc.sbuf_tensor("acc", [P, len(BINS)], f32))
        pss = [
            ctx.enter_context(nc.psum_tensor(f"ps{j}", [P, N], f32)) for j in range(2)
        ]
        dma_sems = [
            ctx.enter_context(nc.semaphore(f"dma{j}")) for j in range(NSLOT)
        ]
        pe_sem = ctx.enter_context(nc.semaphore("pe"))
        act_sem = ctx.enter_context(nc.semaphore("act"))
        odma_sem = ctx.enter_context(nc.semaphore("odma"))
        block = ctx.enter_context(nc.Block())

        # the first few DMAs issue from the (idle until its first exp) ScalarE
        # HWDGE queue so both issue queues run in parallel during the ramp
        SPLITD = 3

        @block.sync
        def _(sync):
            for s, (k, bi, off) in enumerate(seq):
                if s < SPLITD:
                    continue
                i0, m, fd = _tile_geom(k)
                kk = 4 + m
                if s >= NSLOT:
                    # slot reuse: tile s-NSLOT must be consumed by PE first
                    sync.wait_ge(pe_sem, s - NSLOT + 1)
                sync.dma_start(
                    out=mvs[s % NSLOT][0:kk, 0:P + fd],
                    in_=mov[k * P:k * P + kk, 0:P + fd],
                ).then_inc(dma_sems[s % NSLOT], 16)

        @block.tensor
        def _(tensor):
            for s, (k, bi, off) in enumerate(seq):
                i0, m, fd = _tile_geom(k)
                kk = 4 + m
                mv = mvs[s % NSLOT]
                ps = pss[bi % 2]
                tensor.wait_ge(dma_sems[s % NSLOT], 16 * (s // NSLOT + 1))
                if off == 0 and bi >= 2:
                    # PSUM half ping-pong: exp of bin bi-2 must have read it
                    tensor.wait_ge(act_sem, bi - 1)
                # chunk on absolute psum columns, split at 512 bank boundaries
                c0 = off
                while c0 < off + fd:
                    c1 = min(off + fd, (c0 // CHUNK + 1) * CHUNK)
                    mm = tensor.matmul(
                        ps[0:P, c0:c1],
                        lhsT=mv[0:kk, 0:P],
                        rhs=mv[0:kk, P + (c0 - off):P + (c1 - off)],
                        start=True, stop=True,
                    )
                    c0 = c1
                mm.then_inc(pe_sem, 1)

        @block.scalar
        def _(scalar):
            for s in range(SPLITD):
                k, bi, off = seq[s]
                i0, m, fd = _tile_geom(k)
                kk = 4 + m
                scalar.dma_start(
                    out=mvs[s % NSLOT][0:kk, 0:P + fd],
                    in_=mov[k * P:k * P + kk, 0:P + fd],
                ).then_inc(dma_sems[s % NSLOT], 16)
            for bi, tks in enumerate(BINS):
                binw = sum(_tile_geom(k)[2] for k in tks)
                ps = pss[bi % 2]
                scalar.wait_ge(pe_sem, tiles_through_bin[bi])
                scalar.activation(
                    out=ps[0:P, 0:binw], in_=ps[0:P, 0:binw],
                    func=mybir.ActivationFunctionType.Exp,
                    scale=-1.0 / TEMPERATURE,
                    accum_out=acc[0:P, bi:bi + 1],
                ).then_inc(act_sem, 1)
            # act_sem increments at instruction *completion*; without this wait
            # the DMA could read acc before the last accum write lands in SBUF
            scalar.wait_ge(act_sem, len(BINS))
            scalar.dma_start(out=out[:, :], in_=acc[:, :]).then_inc(odma_sem, 16)
            scalar.wait_ge(odma_sem, 16)

    nc.compile()
    return nc


_TRIU = None


def _host_prep(embedding, abs_coords, patch_mask):
    global _TRIU
    if _TRIU is None:
        _TRIU = np.triu(np.ones((N, N), dtype=bool), k=1)

    pos = embedding.astype(np.float64) + abs_coords.astype(np.float64)  # [B,N,D]
    x = pos[:, :, 0]
    y = pos[:, :, 1]
    r = x * x + y * y
    ones = np.ones_like(x)
    qt_all = np.stack([x, y, r, ones], axis=1).astype(np.float16)          # [B,4,N]
    kt_all = np.stack([-2.0 * x, -2.0 * y, ones, r], axis=1).astype(np.float16)

    eye = np.eye(MT, dtype=np.float16)
    in_maps = []
    for b in range(B):
        mb = patch_mask[b] == 1
        msum = mb.astype(np.int8) + mb.astype(np.int8).T
        pen = np.where(msum == 2, 0.0, np.where(msum == 1, LN2T, BIG))
        pen = np.where(_TRIU, pen, BIG).astype(np.float16)

        mov_b = np.zeros((NTILES * P, MOVW), dtype=np.float16)
        for k in range(NTILES):
            i0, m, fd = _tile_geom(k)
            blk = mov_b[k * P:k * P + 4 + m]
            blk[0:4, 0:m] = qt_all[b][:, i0:i0 + m]          # stationary: Q
            blk[4:4 + m, 0:m] = eye[0:m, 0:m]                # stationary: identity
            # dummy output rows m..127: [0,0,BIG,0] . [.,.,1,.] = BIG -> exp 0
            blk[2, m:P] = BIG
            blk[0:4, P:P + fd] = kt_all[b][:, i0:N]          # moving: K
            blk[4:4 + m, P:P + fd] = pen[i0:i0 + m, i0:N]    # moving: penalties
        in_maps.append({"mov": mov_b})
    return in_maps


def kernel(embedding, abs_coords, patch_mask):
    global LAST_RESULTS
    from concourse.bass_utils import run_bass_kernel_spmd

    embedding = np.asarray(embedding)
    abs_coords = np.asarray(abs_coords)
    patch_mask = np.asarray(patch_mask)

    if "nc" not in _cache:
        _cache["nc"] = _build()
    nc = _cache["nc"]

    in_maps = _host_prep(embedding, abs_coords, patch_mask)

    res = run_bass_kernel_spmd(
        nc, in_maps, core_ids=list(range(B)),
        trace=TRACE, trace_cores=[0] if TRACE else None,
    )
    LAST_RESULTS = res

    s_hw = sum(res.results[b]["out"].astype(np.float64).sum() for b in range(B))
    count = np.count_nonzero(patch_mask == 1)
    diag_cnt = sum(
        int(np.trace((patch_mask[b] == 1).astype(np.int64))) for b in range(B)
    )
    loss = np.float64(count) - 2.0 * s_hw - np.float64(diag_cnt)
    return np.array(loss, dtype=np.float32)



# revision 9
# speedup vs baseline: 1.2079x; 1.2079x over previous
"""Distributed Trainium2 (Bass) kernel for nn_AnchorLoss — polynomial-feature version.

Reference:
  pos  = embedding + abs_coords                     [B, N, D],  B=8, N=2048, D=2
  sq   = ||pos_i - pos_j||^2                        [B, N, N]
  loss = sum over (b,i,j) with patch_mask==1 of (1 - exp(-sq / T))

Distribution: batch b -> NeuronCore b (8 cores, data parallel); host combines
the per-core partial sums (scalar all-reduce is free host-side).

Math (per core). With E_ij = exp(-sq_ij/T) (symmetric, E_ii = 1):
  loss_b = count(mask==1) - diag(mask) - T_b,
  T_b    = sum_{i<j} msum_ij E_ij,   msum = mask + mask^T in {0,1,2}.
The Gaussian kernel factorizes exactly through a degree-8 polynomial feature
map (Taylor of exp(2 p_i.p_j / T), AM-GM-damped by exp(-(r_i+r_j)/T), so the
truncation tail is bounded by e^{-a} a^9/9! ~ 1e-5 rel):
  E_ij ~= sum_f v_f[i] v_f[j],  f = (k,t), k<=8, t<=k  ->  F = 45 features
  v_(k,t)[i] = exp(-r_i/T) sqrt((2/T)^k C(k,t)/k!) x_i^t y_i^(k-t)
Then T_b = sum_f v_f^T W v_f with W = triu(msum, 1) -- NO on-device exp at all
(the baseline burned ~15us on ScalarE exp + 2.7us act-table load).

Kernel (per core):
  W is fp8_e4m3 ({0,1,2} exact); V is fp8 hi+lo (two rows per feature: v ~=
  vh+vl, quantization ~0.4% -> verified 1.3e-5 end-to-end rel err in numpy).
  Row-chunk k (i in [128k,128k+128)) covers block-upper-triangle cols
  j in [128k, 2048).  One PE pass: matmul(lhsT=V_chunk [128,90] fp8,
  rhs=W_chunk [128, 2048-128k] fp8) accumulating CT[f, j] = sum_i v_f[i] W_ij
  into a single PSUM region [90, 2048] f32 (4 banks), split at 512-col bank
  boundaries (start only on chunk 0; stop on each bank's last writer 4b+3).
  PSUM bank b is final after chunk 4b+3, so VectorE overlaps the tail:
  tensor_tensor_reduce (fused multiply+row-reduce, one DVE op per bank):
    accum[f, b] = sum_j CT[f, j] * U[f, j],  U = f16 features [90, 2048].
  Total per-core DMA ~2.8 MB (vs 5.15 MB) and PE stream 17408 cols.
  DMA: per-chunk transfers (V packed beside W in one DRAM row-block), split
  across the two HWDGE rings (sync + scalar queues) for issue overlap; no
  slot reuse -- every chunk has its own SBUF home (18.8 KB/partition total).
  Host sums acc [90, 4] per core in float64.
"""

from contextlib import ExitStack
from math import comb, factorial

import numpy as np
import ml_dtypes

B, N, D = 8, 2048, 2
TEMPERATURE = 10.0
P = 128
NCHUNK = N // P               # 16 row chunks of 128
KDEG = 8
F = (KDEG + 1) * (KDEG + 2) // 2   # 45
F2 = 2 * F                         # 90 (hi+lo rows)
CHUNKW = [90 + (N - P * k) for k in range(NCHUNK)]
OFF = np.cumsum([0] + CHUNKW).tolist()   # segment offsets in the mov buffer
MOVW = OFF[-1]                            # 18848
NBANK = 4                                 # 512-col f32 PSUM banks in [0, 2048)
FP8 = ml_dtypes.float8_e4m3

TRACE = False        # set True (see test.py) to neuron-profile the run
LAST_RESULTS = None  # BassKernelResults of the last run when TRACE

_cache = {}


def _build():
    from concourse import bacc, mybir

    nc = bacc.Bacc(enable_partition_id=False)
    f32 = mybir.dt.float32
    f16 = mybir.dt.float16
    f8 = mybir.dt.float8e4
    mov = nc.declare_dram_parameter("mov", [P, MOVW], f8, isOutput=False)
    u = nc.declare_dram_parameter("u", [F2, N], f16, isOutput=False)
    out = nc.declare_dram_parameter("out", [F2, NBANK], f32, isOutput=True)

    # DMA groups of chunks: one dma_start + one semaphore each (a wait on an
    # intermediate count of a shared sem is racy -- per-engine sub-DMA
    # completions from different transfers interleave). Sizes ramp so the PE
    # starts early while later groups stream ahead of consumption.
    GROUPS = [[0], [1], [2, 3], [4, 5, 6, 7], [8, 9, 10, 11, 12, 13, 14, 15]]
    group_of = {k: g for g, ks in enumerate(GROUPS) for k in ks}

    with ExitStack() as ctx:
        big = ctx.enter_context(nc.sbuf_tensor("big", [P, MOVW], f8))
        u_sb = ctx.enter_context(nc.sbuf_tensor("u_sb", [F2, N], f16))
        scratch = ctx.enter_context(nc.sbuf_tensor("scratch", [F2, N], f32))
        wrm = ctx.enter_context(nc.sbuf_tensor("wrm", [F2, 128], f32))
        acc = ctx.enter_context(nc.sbuf_tensor("acc", [F2, NBANK], f32))
        ps = ctx.enter_context(nc.psum_tensor("ps", [P, N], f32))
        gsems = [
            ctx.enter_context(nc.semaphore(f"gsem{g}")) for g in range(len(GROUPS))
        ]
        usem = ctx.enter_context(nc.semaphore("usem"))
        wsem = ctx.enter_context(nc.semaphore("wsem"))
        msem = ctx.enter_context(nc.semaphore("msem"))
        pe_sem = ctx.enter_context(nc.semaphore("pe"))
        dve_sem = ctx.enter_context(nc.semaphore("dve"))
        osem = ctx.enter_context(nc.semaphore("osem"))
        block = ctx.enter_context(nc.Block())

        @block.sync
        def _(sync):
            for g, ks in enumerate(GROUPS):
                sync.dma_start(
                    out=big[0:P, OFF[ks[0]]:OFF[ks[-1] + 1]],
                    in_=mov[0:P, OFF[ks[0]]:OFF[ks[-1] + 1]],
                ).then_inc(gsems[g], 16)
            sync.wait_ge(dve_sem, NBANK)
            sync.dma_start(out=out[:, :], in_=acc[:, :]).then_inc(osem, 16)
            sync.wait_ge(osem, 16)

        @block.scalar
        def _(scalar):
            # U streams on the second HWDGE ring, deferred behind group 0 so
            # the PE-critical first chunk gets the full DMA bandwidth.
            scalar.wait_ge(gsems[0], 16)
            scalar.dma_start(out=u_sb[:, :], in_=u[:, :]).then_inc(usem, 16)

        @block.tensor
        def _(tensor):
            for k in range(NCHUNK):
                if k == GROUPS[group_of[k]][0]:
                    tensor.wait_ge(gsems[group_of[k]], 16)
                lhsT = big[0:P, OFF[k]:OFF[k] + F2]
                wbase = OFF[k] + F2
                c0 = P * k
                mm = None
                while c0 < N:
                    c1 = min(N, (c0 // 512 + 1) * 512)
                    bank = c0 // 512
                    mm = tensor.matmul(
                        ps[0:F2, c0:c1],
                        lhsT=lhsT,
                        rhs=big[0:P, wbase + (c0 - P * k):wbase + (c1 - P * k)],
                        start=(k == 0),
                        stop=(k == 4 * bank + 3),
                    )
                    c0 = c1
                mm.then_inc(pe_sem, 1)

        @block.vector
        def _(vector):
            for b in range(NBANK):
                vector.wait_ge(pe_sem, 4 * b + 4)
                if b == 0:
                    vector.wait_ge(usem, 16)  # U resident
                vector.tensor_mul(
                    scratch[0:F2, 512 * b:512 * (b + 1)],
                    ps[0:F2, 512 * b:512 * (b + 1)],
                    u_sb[0:F2, 512 * b:512 * (b + 1)],
                ).then_inc(msem, 1)
                vector.wait_ge(msem, b + 1)
                vector.tensor_reduce(
                    acc[0:F2, b:b + 1],
                    scratch[0:F2, 512 * b:512 * (b + 1)],
                    axis=mybir.AxisListType.X,
                    op=mybir.AluOpType.add,
                ).then_inc(dve_sem, 1)

    nc.compile()
    return nc


_TRIU128 = None


def _features(pos):
    """pos [B, N, 2] float64 -> V [B, N, F] float64."""
    x, y = pos[:, :, 0], pos[:, :, 1]
    r = x * x + y * y
    damp = np.exp(-r / TEMPERATURE)
    xp = [np.ones_like(x)]
    yp = [np.ones_like(y)]
    for _ in range(KDEG):
        xp.append(xp[-1] * x)
        yp.append(yp[-1] * y)
    cols = []
    for k in range(KDEG + 1):
        for t in range(k + 1):
            c = np.sqrt((2.0 / TEMPERATURE) ** k * comb(k, t) / factorial(k))
            cols.append(damp * c * xp[t] * yp[k - t])
    return np.stack(cols, axis=2)


def _host_prep(embedding, abs_coords, patch_mask):
    global _TRIU128
    if _TRIU128 is None:
        _TRIU128 = np.triu(np.ones((P, P), dtype=np.uint8), k=1)

    pos = embedding.astype(np.float64) + abs_coords.astype(np.float64)
    V = _features(pos)                                   # [B, N, F] f64
    Vh = V.astype(FP8)
    Vl = (V - Vh.astype(np.float64)).astype(FP8)
    Vh_u8 = Vh.view(np.uint8)
    Vl_u8 = Vl.view(np.uint8)
    U = V.astype(np.float16)                             # [B, N, F]

    lut = np.array([0.0, 1.0, 2.0], dtype=FP8).view(np.uint8)  # msum -> fp8 byte

    in_maps = []
    for b in range(B):
        mb = (patch_mask[b] == 1).astype(np.uint8)
        mov_b = np.empty((P, MOVW), dtype=np.uint8)
        for k in range(NCHUNK):
            o = OFF[k]
            rs = slice(P * k, P * (k + 1))
            mov_b[:, o:o + F] = Vh_u8[b, rs]
            mov_b[:, o + F:o + F2] = Vl_u8[b, rs]
            msum = mb[rs, P * k:] + mb[P * k:, rs].T     # [128, W_k] in {0,1,2}
            msum[:, :P] *= _TRIU128                      # strict upper in diag block
            mov_b[:, o + F2:OFF[k + 1]] = lut[msum]
        u_b = np.empty((F2, N), dtype=np.float16)
        u_b[:F] = U[b].T
        u_b[F:] = U[b].T
        in_maps.append({"mov": mov_b.view(FP8), "u": u_b})
    return in_maps


def kernel(embedding, abs_coords, patch_mask):
    global LAST_RESULTS
    from concourse.bass_utils import run_bass_kernel_spmd

    embedding = np.asarray(embedding)
    abs_coords = np.asarray(abs_coords)
    patch_mask = np.asarray(patch_mask)

    if "nc" not in _cache:
        _cache["nc"] = _build()
    nc = _cache["nc"]

    in_maps = _host_prep(embedding, abs_coords, patch_mask)

    res = run_bass_kernel_spmd(
        nc, in_maps, core_ids=list(range(B)),
        trace=TRACE, trace_cores=[0] if TRACE else None,
    )
    LAST_RESULTS = res

    t_hw = sum(res.results[b]["out"].astype(np.float64).sum() for b in range(B))
    count = np.count_nonzero(patch_mask == 1)
    diag_cnt = sum(
        int(np.trace((patch_mask[b] == 1).astype(np.int64))) for b in range(B)
    )
    loss = np.float64(count) - np.float64(diag_cnt) - t_hw
    return np.array(loss, dtype=np.float32)


# revision 10
# speedup vs baseline: 1.3248x; 1.0968x over previous
"""Distributed Trainium2 (Bass) kernel for nn_AnchorLoss — polynomial-feature version.

Reference:
  pos  = embedding + abs_coords                     [B, N, D],  B=8, N=2048, D=2
  sq   = ||pos_i - pos_j||^2                        [B, N, N]
  loss = sum over (b,i,j) with patch_mask==1 of (1 - exp(-sq / T))

Distribution: batch b -> NeuronCore b (8 cores, data parallel); host combines
the per-core partial sums (scalar all-reduce is free host-side).

Math (per core). With E_ij = exp(-sq_ij/T) (symmetric, E_ii = 1):
  loss_b = count(mask==1) - diag(mask) - T_b,
  T_b    = sum_{i<j} msum_ij E_ij,   msum = mask + mask^T in {0,1,2}.
The Gaussian kernel factorizes exactly through a degree-8 polynomial feature
map (Taylor of exp(2 p_i.p_j / T); |2 p.q| <= r_i + r_j so the truncation
tail is damped by exp(-(r_i+r_j)/T) -> ~1e-5 end-to-end):
  E_ij ~= sum_f v_f[i] v_f[j],  f = (k,t), k<=8, t<=k  ->  F = 45 features
  v_(k,t)[i] = exp(-r_i/T) sqrt((2/T)^k C(k,t)/k!) x_i^t y_i^(k-t)
Then T_b = sum_f v_f^T W v_f with W = triu(msum, 1) -- NO on-device exp at all
(the baseline burned ~15us of ScalarE exp + a 2.7us act-table load on it).

Kernel (per core):
  W is fp8_e4m3 ({0,1,2} exact); V is fp8 hi+lo (v ~= vh+vl, ~0.4% quant).
  Row-chunk k (i in [128k,128k+128)) covers block-upper-triangle cols
  j in [128k, 2048): matmul(lhsT=V_chunk [128,90] fp8, rhs=W_chunk fp8)
  accumulates CT[f, j] = sum_i v_f[i] W_ij into one PSUM region [90, 2048]
  (split at 512-col bank bounds; start on chunk 0, stop on bank's last
  writer). Bank b finalizes after chunk 4b+3, so the DVE overlaps the tail:
  tensor_mul (PSUM x U -> scratch) + tensor_reduce -> acc column, U = f16
  features (tensor_tensor_reduce would fuse these but hangs TRN2 hardware).
  DMA: chunks coalesced into 6 size-ramped groups, each its own contiguous
  DRAM parameter (row-major [128, W_g]: sequential HBM bursts instead of
  2 KB strided lines) and its own semaphore (a wait on an intermediate count
  of a shared DMA sem is racy: per-engine sub-DMA completions interleave).
  Issues split across both HWDGE rings (sync + scalar) to overlap the
  ~0.7 us per-dma_start issue cost. While group 0 flies, the PE runs junk
  f16 matmuls into PSUM rows that chunk 0 later overwrites -- the HAM
  clock-gate sees a busy PE and un-throttles 1.2->2.4 GHz before real work.
  Output acc is DMA'd to DRAM in two pieces so the HBM write receipt of the
  first overlaps the last bank's reduce. Host sums acc [90, 5] in float64.
"""

from contextlib import ExitStack
from math import comb, factorial

import numpy as np
import ml_dtypes

B, N, D = 8, 2048, 2
TEMPERATURE = 10.0
P = 128
NCHUNK = N // P               # 16 row chunks of 128
KDEG = 8
F = (KDEG + 1) * (KDEG + 2) // 2   # 45
F2 = 2 * F                         # 90 (hi+lo rows)
CHUNKW = [F2 + (N - P * k) for k in range(NCHUNK)]
OFF = np.cumsum([0] + CHUNKW).tolist()   # chunk offsets in the SBUF buffer
MOVW = OFF[-1]                            # 18848
FP8 = ml_dtypes.float8_e4m3

# DMA groups of chunks (one contiguous DRAM param + one semaphore + one
# dma_start each); sizes ramp so the PE starts early and prefetch stays ahead
GROUPS = [[0], [1], [2, 3], [4, 5, 6, 7], [8, 9, 10, 11], [12, 13, 14, 15]]
GW = [sum(CHUNKW[k] for k in ks) for ks in GROUPS]
# DVE work items: (psum col range, pe_sem threshold, acc col)
DVE_ITEMS = [
    (0, 512, 4, 0),
    (512, 1024, 8, 1),
    (1024, 1536, 12, 2),
    (1536, 1920, 15, 3),
    (1920, 2048, 16, 4),
]
NACC = len(DVE_ITEMS)

TRACE = False        # set True (see test.py) to neuron-profile the run
LAST_RESULTS = None  # BassKernelResults of the last run when TRACE

_cache = {}


def _build():
    from concourse import bacc, mybir

    nc = bacc.Bacc(enable_partition_id=False)
    f32 = mybir.dt.float32
    f16 = mybir.dt.float16
    f8 = mybir.dt.float8e4
    movs = [
        nc.declare_dram_parameter(f"mov{g}", [P, GW[g]], f8, isOutput=False)
        for g in range(len(GROUPS))
    ]
    u = nc.declare_dram_parameter("u", [F2, N], f16, isOutput=False)
    out = nc.declare_dram_parameter("out", [F2, NACC], f32, isOutput=True)

    group_of = {k: g for g, ks in enumerate(GROUPS) for k in ks}

    with ExitStack() as ctx:
        big = ctx.enter_context(nc.sbuf_tensor("big", [P, MOVW], f8))
        u_sb = ctx.enter_context(nc.sbuf_tensor("u_sb", [F2, N], f16))
        scratch = ctx.enter_context(nc.sbuf_tensor("scratch", [F2, N], f32))
        wrm = ctx.enter_context(nc.sbuf_tensor("wrm", [P, 512], f16))
        acc = ctx.enter_context(nc.sbuf_tensor("acc", [F2, NACC], f32))
        ps = ctx.enter_context(nc.psum_tensor("ps", [P, N], f32))
        gsems = [
            ctx.enter_context(nc.semaphore(f"gsem{g}")) for g in range(len(GROUPS))
        ]
        usem = ctx.enter_context(nc.semaphore("usem"))
        wsem = ctx.enter_context(nc.semaphore("wsem"))
        msem = ctx.enter_context(nc.semaphore("msem"))
        pe_sem = ctx.enter_context(nc.semaphore("pe"))
        dve_sem = ctx.enter_context(nc.semaphore("dve"))
        osem = ctx.enter_context(nc.semaphore("osem"))
        block = ctx.enter_context(nc.Block())

        def group_dma(eng, g):
            ks = GROUPS[g]
            eng.dma_start(
                out=big[0:P, OFF[ks[0]]:OFF[ks[-1] + 1]],
                in_=movs[g][0:P, 0:GW[g]],
            ).then_inc(gsems[g], 16)

        @block.sync
        def _(sync):
            for g in (0, 1, 2, 4):
                group_dma(sync, g)
            sync.wait_ge(dve_sem, 3)
            sync.dma_start(out=out[:, 0:3], in_=acc[:, 0:3]).then_inc(osem, 16)
            sync.wait_ge(dve_sem, NACC)
            sync.dma_start(out=out[:, 3:NACC], in_=acc[:, 3:NACC]).then_inc(osem, 16)
            sync.wait_ge(osem, 32)

        @block.scalar
        def _(scalar):
            # second HWDGE ring: deferred behind group 0 so the PE-critical
            # first chunk gets the full DMA bandwidth, then prefetches deep
            scalar.wait_ge(gsems[0], 16)
            for g in (3, 5):
                group_dma(scalar, g)
            scalar.dma_start(out=u_sb[:, :], in_=u[:, :]).then_inc(usem, 16)

        @block.tensor
        def _(tensor):
            # HAM warm-up: junk f16 matmuls into rows that chunk 0 later
            # overwrites with start=True; busies the PE during group 0's DMA
            # so the 2.4 GHz un-throttle lands before the real stream.
            tensor.wait_ge(wsem, 1)
            for w in range(4):
                tensor.matmul(
                    ps[0:32, 0:512],
                    lhsT=wrm[0:P, 0:32],
                    rhs=wrm[0:P, 0:512],
                    start=True,
                    stop=True,
                )
            for k in range(NCHUNK):
                if k == GROUPS[group_of[k]][0]:
                    tensor.wait_ge(gsems[group_of[k]], 16)
                lhsT = big[0:P, OFF[k]:OFF[k] + F2]
                wbase = OFF[k] + F2
                c0 = P * k
                mm = None
                while c0 < N:
                    c1 = min(N, (c0 // 512 + 1) * 512)
                    bank = c0 // 512
                    mm = tensor.matmul(
                        ps[0:F2, c0:c1],
                        lhsT=lhsT,
                        rhs=big[0:P, wbase + (c0 - P * k):wbase + (c1 - P * k)],
                        start=(k == 0),
                        stop=(k == 4 * bank + 3),
                    )
                    c0 = c1
                mm.then_inc(pe_sem, 1)

        @block.vector
        def _(vector):
            vector.memset(wrm[0:P, 0:512], 0.0).then_inc(wsem, 1)
            for i, (c0, c1, thr, col) in enumerate(DVE_ITEMS):
                vector.wait_ge(pe_sem, thr)
                if i == 0:
                    vector.wait_ge(usem, 16)  # U resident
                vector.tensor_mul(
                    scratch[0:F2, c0:c1],
                    ps[0:F2, c0:c1],
                    u_sb[0:F2, c0:c1],
                ).then_inc(msem, 1)
                vector.wait_ge(msem, i + 1)
                vector.tensor_reduce(
                    acc[0:F2, col:col + 1],
                    scratch[0:F2, c0:c1],
                    axis=mybir.AxisListType.X,
                    op=mybir.AluOpType.add,
                ).then_inc(dve_sem, 1)

    nc.compile()
    return nc


_TRIU128 = None


def _features(pos):
    """pos [B, N, 2] float64 -> V [B, N, F] float64."""
    x, y = pos[:, :, 0], pos[:, :, 1]
    r = x * x + y * y
    damp = np.exp(-r / TEMPERATURE)
    xp = [np.ones_like(x)]
    yp = [np.ones_like(y)]
    for _ in range(KDEG):
        xp.append(xp[-1] * x)
        yp.append(yp[-1] * y)
    cols = []
    for k in range(KDEG + 1):
        for t in range(k + 1):
            c = np.sqrt((2.0 / TEMPERATURE) ** k * comb(k, t) / factorial(k))
            cols.append(damp * c * xp[t] * yp[k - t])
    return np.stack(cols, axis=2)


def _host_prep(embedding, abs_coords, patch_mask):
    global _TRIU128
    if _TRIU128 is None:
        _TRIU128 = np.triu(np.ones((P, P), dtype=np.uint8), k=1)

    pos = embedding.astype(np.float64) + abs_coords.astype(np.float64)
    V = _features(pos)                                   # [B, N, F] f64
    Vh = V.astype(FP8)
    Vl = (V - Vh.astype(np.float64)).astype(FP8)
    Vh_u8 = Vh.view(np.uint8)
    Vl_u8 = Vl.view(np.uint8)
    U = V.astype(np.float16)                             # [B, N, F]

    lut = np.array([0.0, 1.0, 2.0], dtype=FP8).view(np.uint8)  # msum -> fp8 byte

    in_maps = []
    for b in range(B):
        mb = (patch_mask[b] == 1).astype(np.uint8)
        im = {}
        for g, ks in enumerate(GROUPS):
            mg = np.empty((P, GW[g]), dtype=np.uint8)
            o = 0
            for k in ks:
                rs = slice(P * k, P * (k + 1))
                mg[:, o:o + F] = Vh_u8[b, rs]
                mg[:, o + F:o + F2] = Vl_u8[b, rs]
                msum = mb[rs, P * k:] + mb[P * k:, rs].T   # [128, W_k] in {0,1,2}
                msum[:, :P] *= _TRIU128                    # strict upper, diag block
                mg[:, o + F2:o + CHUNKW[k]] = lut[msum]
                o += CHUNKW[k]
            im[f"mov{g}"] = mg.view(FP8)
        u_b = np.empty((F2, N), dtype=np.float16)
        u_b[:F] = U[b].T
        u_b[F:] = U[b].T
        im["u"] = u_b
        in_maps.append(im)
    return in_maps


def kernel(embedding, abs_coords, patch_mask):
    global LAST_RESULTS
    from concourse.bass_utils import run_bass_kernel_spmd

    embedding = np.asarray(embedding)
    abs_coords = np.asarray(abs_coords)
    patch_mask = np.asarray(patch_mask)

    if "nc" not in _cache:
        _cache["nc"] = _build()
    nc = _cache["nc"]

    in_maps = _host_prep(embedding, abs_coords, patch_mask)

    res = run_bass_kernel_spmd(
        nc, in_maps, core_ids=list(range(B)),
        trace=TRACE, trace_cores=[0] if TRACE else None,
    )
    LAST_RESULTS = res

    t_hw = sum(res.results[b]["out"].astype(np.float64).sum() for b in range(B))
    count = np.count_nonzero(patch_mask == 1)
    diag_cnt = sum(
        int(np.trace((patch_mask[b] == 1).astype(np.int64))) for b in range(B)
    )
    loss = np.float64(count) - np.float64(diag_cnt) - t_hw
    return np.array(loss, dtype=np.float32)


# revision 11
# speedup vs baseline: 1.3722x; 1.0358x over previous
"""Distributed Trainium2 (Bass) kernel for nn_AnchorLoss — polynomial-feature version.

Reference:
  pos  = embedding + abs_coords                     [B, N, D],  B=8, N=2048, D=2
  sq   = ||pos_i - pos_j||^2                        [B, N, N]
  loss = sum over (b,i,j) with patch_mask==1 of (1 - exp(-sq / T))

Distribution: batch b -> NeuronCore b (8 cores, data parallel); host combines
the per-core partial sums (scalar all-reduce is free host-side).

Math (per core). With E_ij = exp(-sq_ij/T) (symmetric, E_ii = 1):
  loss_b = count(mask==1) - diag(mask) - T_b,
  T_b    = sum_{i<j} msum_ij E_ij,   msum = mask + mask^T in {0,1,2}.
The Gaussian kernel factorizes exactly through a degree-8 polynomial feature
map (Taylor of exp(2 p_i.p_j / T); |2 p.q| <= r_i + r_j so the truncation
tail is damped by exp(-(r_i+r_j)/T) -> ~1e-5 end-to-end):
  E_ij ~= sum_f v_f[i] v_f[j],  f = (k,t), k<=8, t<=k  ->  F = 45 features
  v_(k,t)[i] = exp(-r_i/T) sqrt((2/T)^k C(k,t)/k!) x_i^t y_i^(k-t)
Then T_b = sum_f v_f^T W v_f with W = triu(msum, 1) -- NO on-device exp at all
(the baseline burned ~15us of ScalarE exp + a 2.7us act-table load on it).

Kernel (per core):
  W is fp8_e4m3 ({0,1,2} exact); V is fp8 hi+lo (v ~= vh+vl, ~0.4% quant).
  Row-chunk k (i in [128k,128k+128)) covers block-upper-triangle cols
  j in [128k, 2048): matmul(lhsT=V_chunk [128,90] fp8, rhs=W_chunk fp8)
  accumulates CT[f, j] = sum_i v_f[i] W_ij into one PSUM region [90, 2048]
  (split at 512-col bank bounds; start on chunk 0, stop on bank's last
  writer). Bank b finalizes after chunk 4b+3, so the DVE overlaps the tail:
  tensor_mul (PSUM x U -> scratch) + tensor_reduce -> acc column, U = f16
  features (tensor_tensor_reduce would fuse these but hangs TRN2 hardware).
  DMA: chunks coalesced into 6 size-ramped groups, each its own contiguous
  DRAM parameter (row-major [128, W_g]: sequential HBM bursts instead of
  2 KB strided lines) and its own semaphore (a wait on an intermediate count
  of a shared DMA sem is racy: per-engine sub-DMA completions interleave).
  Issues split across both HWDGE rings (sync + scalar) to overlap the
  ~0.7 us per-dma_start issue cost. While group 0 flies, the PE runs junk
  f16 matmuls into PSUM rows that chunk 0 later overwrites -- the HAM
  clock-gate sees a busy PE and un-throttles 1.2->2.4 GHz before real work.
  Output acc is DMA'd to DRAM in two pieces so the HBM write receipt of the
  first overlaps the last bank's reduce. Host sums acc [90, 5] in float64.
"""

from contextlib import ExitStack
from math import comb, factorial

import numpy as np
import ml_dtypes

B, N, D = 8, 2048, 2
TEMPERATURE = 10.0
P = 128
NCHUNK = N // P               # 16 row chunks of 128
KDEG = 8
F = (KDEG + 1) * (KDEG + 2) // 2   # 45
F2 = 2 * F                         # 90 (hi+lo rows)
CHUNKW = [F2 + (N - P * k) for k in range(NCHUNK)]
OFF = np.cumsum([0] + CHUNKW).tolist()   # chunk offsets in the SBUF buffer
MOVW = OFF[-1]                            # 18848
FP8 = ml_dtypes.float8_e4m3

# DMA groups of chunks (one contiguous DRAM param + one semaphore + one
# dma_start each); sizes ramp so the PE starts early and prefetch stays ahead
GROUPS = [[0], [1], [2, 3], [4, 5, 6, 7], [8, 9, 10, 11], [12, 13, 14, 15]]
GW = [sum(CHUNKW[k] for k in ks) for ks in GROUPS]
# DVE work items: (psum col range, pe_sem threshold, acc col)
DVE_ITEMS = [
    (0, 512, 4, 0),
    (512, 1024, 8, 1),
    (1024, 1536, 12, 2),
    (1536, 1920, 15, 3),
    (1920, 2048, 16, 4),
]
NACC = len(DVE_ITEMS)

TRACE = False        # set True (see test.py) to neuron-profile the run
LAST_RESULTS = None  # BassKernelResults of the last run when TRACE

_cache = {}


def _build():
    from concourse import bacc, mybir

    nc = bacc.Bacc(enable_partition_id=False)
    f32 = mybir.dt.float32
    f16 = mybir.dt.float16
    f8 = mybir.dt.float8e4
    movs = [
        nc.declare_dram_parameter(f"mov{g}", [P, GW[g]], f8, isOutput=False)
        for g in range(len(GROUPS))
    ]
    u = nc.declare_dram_parameter("u", [F2, N], f16, isOutput=False)
    out = nc.declare_dram_parameter("out", [F2, NACC], f32, isOutput=True)

    group_of = {k: g for g, ks in enumerate(GROUPS) for k in ks}

    with ExitStack() as ctx:
        big = ctx.enter_context(nc.sbuf_tensor("big", [P, MOVW], f8))
        u_sb = ctx.enter_context(nc.sbuf_tensor("u_sb", [F2, N], f16))
        scratch = ctx.enter_context(nc.sbuf_tensor("scratch", [F2, N], f32))
        wrm = ctx.enter_context(nc.sbuf_tensor("wrm", [P, 512], f16))
        acc = ctx.enter_context(nc.sbuf_tensor("acc", [F2, NACC], f32))
        ps = ctx.enter_context(nc.psum_tensor("ps", [P, N], f32))
        gsems = [
            ctx.enter_context(nc.semaphore(f"gsem{g}")) for g in range(len(GROUPS))
        ]
        usem = ctx.enter_context(nc.semaphore("usem"))
        wsem = ctx.enter_context(nc.semaphore("wsem"))
        msem = ctx.enter_context(nc.semaphore("msem"))
        pe_sem = ctx.enter_context(nc.semaphore("pe"))
        dve_sem = ctx.enter_context(nc.semaphore("dve"))
        osem = ctx.enter_context(nc.semaphore("osem"))
        block = ctx.enter_context(nc.Block())

        def group_dma(eng, g):
            ks = GROUPS[g]
            eng.dma_start(
                out=big[0:P, OFF[ks[0]]:OFF[ks[-1] + 1]],
                in_=movs[g][0:P, 0:GW[g]],
            ).then_inc(gsems[g], 16)

        @block.sync
        def _(sync):
            for g in range(len(GROUPS)):
                group_dma(sync, g)
            sync.wait_ge(dve_sem, 3)
            sync.dma_start(out=out[:, 0:3], in_=acc[:, 0:3]).then_inc(osem, 16)
            sync.wait_ge(dve_sem, NACC)
            sync.dma_start(out=out[:, 3:NACC], in_=acc[:, 3:NACC]).then_inc(osem, 16)
            sync.wait_ge(osem, 32)

        @block.scalar
        def _(scalar):
            # second HWDGE ring: U defers behind the first two PE-critical
            # groups, then streams while the PE chews through them
            scalar.wait_ge(gsems[1], 16)
            scalar.dma_start(out=u_sb[:, :], in_=u[:, :]).then_inc(usem, 16)

        @block.tensor
        def _(tensor):
            # HAM warm-up: junk f16 matmuls into rows that chunk 0 later
            # overwrites with start=True; busies the PE during group 0's DMA
            # so the 2.4 GHz un-throttle lands before the real stream.
            tensor.wait_ge(wsem, 1)
            for w in range(8):
                tensor.matmul(
                    ps[0:32, 0:512],
                    lhsT=wrm[0:P, 0:32],
                    rhs=wrm[0:P, 0:512],
                    start=True,
                    stop=True,
                )
            for k in range(NCHUNK):
                if k == GROUPS[group_of[k]][0]:
                    tensor.wait_ge(gsems[group_of[k]], 16)
                lhsT = big[0:P, OFF[k]:OFF[k] + F2]
                wbase = OFF[k] + F2
                c0 = P * k
                mm = None
                while c0 < N:
                    c1 = min(N, (c0 // 512 + 1) * 512)
                    bank = c0 // 512
                    mm = tensor.matmul(
                        ps[0:F2, c0:c1],
                        lhsT=lhsT,
                        rhs=big[0:P, wbase + (c0 - P * k):wbase + (c1 - P * k)],
                        start=(k == 0),
                        stop=(k == 4 * bank + 3),
                    )
                    c0 = c1
                mm.then_inc(pe_sem, 1)

        @block.vector
        def _(vector):
            vector.memset(wrm[0:P, 0:512], 0.0).then_inc(wsem, 1)
            for i, (c0, c1, thr, col) in enumerate(DVE_ITEMS):
                vector.wait_ge(pe_sem, thr)
                if i == 0:
                    vector.wait_ge(usem, 16)  # U resident
                vector.tensor_mul(
                    scratch[0:F2, c0:c1],
                    ps[0:F2, c0:c1],
                    u_sb[0:F2, c0:c1],
                ).then_inc(msem, 1)
                vector.wait_ge(msem, i + 1)
                vector.tensor_reduce(
                    acc[0:F2, col:col + 1],
                    scratch[0:F2, c0:c1],
                    axis=mybir.AxisListType.X,
                    op=mybir.AluOpType.add,
                ).then_inc(dve_sem, 1)

    nc.compile()
    return nc


_TRIU128 = None


def _features(pos):
    """pos [B, N, 2] float64 -> V [B, N, F] float64."""
    x, y = pos[:, :, 0], pos[:, :, 1]
    r = x * x + y * y
    damp = np.exp(-r / TEMPERATURE)
    xp = [np.ones_like(x)]
    yp = [np.ones_like(y)]
    for _ in range(KDEG):
        xp.append(xp[-1] * x)
        yp.append(yp[-1] * y)
    cols = []
    for k in range(KDEG + 1):
        for t in range(k + 1):
            c = np.sqrt((2.0 / TEMPERATURE) ** k * comb(k, t) / factorial(k))
            cols.append(damp * c * xp[t] * yp[k - t])
    return np.stack(cols, axis=2)


def _host_prep(embedding, abs_coords, patch_mask):
    global _TRIU128
    if _TRIU128 is None:
        _TRIU128 = np.triu(np.ones((P, P), dtype=np.uint8), k=1)

    pos = embedding.astype(np.float64) + abs_coords.astype(np.float64)
    V = _features(pos)                                   # [B, N, F] f64
    Vh = V.astype(FP8)
    Vl = (V - Vh.astype(np.float64)).astype(FP8)
    Vh_u8 = Vh.view(np.uint8)
    Vl_u8 = Vl.view(np.uint8)
    U = V.astype(np.float16)                             # [B, N, F]

    lut = np.array([0.0, 1.0, 2.0], dtype=FP8).view(np.uint8)  # msum -> fp8 byte

    in_maps = []
    for b in range(B):
        mb = (patch_mask[b] == 1).astype(np.uint8)
        im = {}
        for g, ks in enumerate(GROUPS):
            mg = np.empty((P, GW[g]), dtype=np.uint8)
            o = 0
            for k in ks:
                rs = slice(P * k, P * (k + 1))
                mg[:, o:o + F] = Vh_u8[b, rs]
                mg[:, o + F:o + F2] = Vl_u8[b, rs]
                msum = mb[rs, P * k:] + mb[P * k:, rs].T   # [128, W_k] in {0,1,2}
                msum[:, :P] *= _TRIU128                    # strict upper, diag block
                mg[:, o + F2:o + CHUNKW[k]] = lut[msum]
                o += CHUNKW[k]
            im[f"mov{g}"] = mg.view(FP8)
        u_b = np.empty((F2, N), dtype=np.float16)
        u_b[:F] = U[b].T
        u_b[F:] = U[b].T
        im["u"] = u_b
        in_maps.append(im)
    return in_maps


def kernel(embedding, abs_coords, patch_mask):
    global LAST_RESULTS
    from concourse.bass_utils import run_bass_kernel_spmd

    embedding = np.asarray(embedding)
    abs_coords = np.asarray(abs_coords)
    patch_mask = np.asarray(patch_mask)

    if "nc" not in _cache:
        _cache["nc"] = _build()
    nc = _cache["nc"]

    in_maps = _host_prep(embedding, abs_coords, patch_mask)

    res = run_bass_kernel_spmd(
        nc, in_maps, core_ids=list(range(B)),
        trace=TRACE, trace_cores=[0] if TRACE else None,
    )
    LAST_RESULTS = res

    t_hw = sum(res.results[b]["out"].astype(np.float64).sum() for b in range(B))
    count = np.count_nonzero(patch_mask == 1)
    diag_cnt = sum(
        int(np.trace((patch_mask[b] == 1).astype(np.int64))) for b in range(B)
    )
    loss = np.float64(count) - np.float64(diag_cnt) - t_hw
    return np.array(loss, dtype=np.float32)


# revision 12
# speedup vs baseline: 1.3824x; 1.0074x over previous
"""Distributed Trainium2 (Bass) kernel for nn_AnchorLoss — polynomial-feature version.

Reference:
  pos  = embedding + abs_coords                     [B, N, D],  B=8, N=2048, D=2
  sq   = ||pos_i - pos_j||^2                        [B, N, N]
  loss = sum over (b,i,j) with patch_mask==1 of (1 - exp(-sq / T))

Distribution: batch b -> NeuronCore b (8 cores, data parallel); host combines
the per-core partial sums (scalar all-reduce is free host-side).

Math (per core). With E_ij = exp(-sq_ij/T) (symmetric, E_ii = 1):
  loss_b = count(mask==1) - diag(mask) - T_b,
  T_b    = sum_{i<j} msum_ij E_ij,   msum = mask + mask^T in {0,1,2}.
The Gaussian kernel factorizes exactly through a degree-8 polynomial feature
map (Taylor of exp(2 p_i.p_j / T); |2 p.q| <= r_i + r_j so the truncation
tail is damped by exp(-(r_i+r_j)/T) -> ~1e-5 end-to-end):
  E_ij ~= sum_f v_f[i] v_f[j],  f = (k,t), k<=8, t<=k  ->  F = 45 features
  v_(k,t)[i] = exp(-r_i/T) sqrt((2/T)^k C(k,t)/k!) x_i^t y_i^(k-t)
Then T_b = sum_f v_f^T W v_f with W = triu(msum, 1) -- NO on-device exp at all
(the baseline burned ~15us of ScalarE exp + a 2.7us act-table load on it).

Kernel (per core):
  W is fp8_e4m3 ({0,1,2} exact); V is fp8 hi+lo (v ~= vh+vl, ~0.4% quant).
  Row-chunk k (i in [128k,128k+128)) covers block-upper-triangle cols
  j in [128k, 2048): matmul(lhsT=V_chunk [128,90] fp8, rhs=W_chunk fp8)
  accumulates CT[f, j] = sum_i v_f[i] W_ij into one PSUM region [90, 2048]
  (split at 512-col bank bounds; start on chunk 0, stop on bank's last
  writer). Bank b finalizes after chunk 4b+3, so the DVE overlaps the tail:
  tensor_mul (PSUM x U -> scratch) + tensor_reduce -> acc column, U = f16
  features (tensor_tensor_reduce would fuse these but hangs TRN2 hardware).
  DMA: chunks coalesced into 6 size-ramped groups, each its own contiguous
  DRAM parameter (row-major [128, W_g]: sequential HBM bursts instead of
  2 KB strided lines) and its own semaphore (a wait on an intermediate count
  of a shared DMA sem is racy: per-engine sub-DMA completions interleave).
  Issues split across both HWDGE rings (sync + scalar) to overlap the
  ~0.7 us per-dma_start issue cost. While group 0 flies, the PE runs junk
  f16 matmuls into PSUM rows that chunk 0 later overwrites -- the HAM
  clock-gate sees a busy PE and un-throttles 1.2->2.4 GHz before real work.
  Output acc is DMA'd to DRAM in two pieces so the HBM write receipt of the
  first overlaps the last bank's reduce. Host sums acc [90, 5] in float64.
"""

from contextlib import ExitStack
from math import comb, factorial

import numpy as np
import ml_dtypes

B, N, D = 8, 2048, 2
TEMPERATURE = 10.0
P = 128
NCHUNK = N // P               # 16 row chunks of 128
KDEG = 8
F = (KDEG + 1) * (KDEG + 2) // 2   # 45
F2 = 2 * F                         # 90 (hi+lo rows)
CHUNKW = [F2 + (N - P * k) for k in range(NCHUNK)]
OFF = np.cumsum([0] + CHUNKW).tolist()   # chunk offsets in the SBUF buffer
MOVW = OFF[-1]                            # 18848
FP8 = ml_dtypes.float8_e4m3

# DMA groups of chunks (one contiguous DRAM param + one semaphore + one
# dma_start each); sizes ramp so the PE starts early and prefetch stays ahead
GROUPS = [[0], [1], [2, 3], [4, 5, 6, 7], [8, 9, 10, 11], [12, 13, 14, 15]]
GW = [sum(CHUNKW[k] for k in ks) for ks in GROUPS]
# DVE work items: (psum col range, pe_sem threshold, acc col)
DVE_ITEMS = [
    (0, 512, 4, 0),
    (512, 1024, 8, 1),
    (1024, 1536, 12, 2),
    (1536, 1920, 15, 3),
    (1920, 2048, 16, 4),
]
NACC = len(DVE_ITEMS)

TRACE = False        # set True (see test.py) to neuron-profile the run
LAST_RESULTS = None  # BassKernelResults of the last run when TRACE

_cache = {}


def _build():
    from concourse import bacc, mybir

    nc = bacc.Bacc(enable_partition_id=False)
    f32 = mybir.dt.float32
    f16 = mybir.dt.float16
    f8 = mybir.dt.float8e4
    movs = [
        nc.declare_dram_parameter(f"mov{g}", [P, GW[g]], f8, isOutput=False)
        for g in range(len(GROUPS))
    ]
    u = nc.declare_dram_parameter("u", [F, N], f16, isOutput=False)
    out = nc.declare_dram_parameter("out", [F2, NACC], f32, isOutput=True)

    group_of = {k: g for g, ks in enumerate(GROUPS) for k in ks}

    with ExitStack() as ctx:
        big = ctx.enter_context(nc.sbuf_tensor("big", [P, MOVW], f8))
        u_sb = ctx.enter_context(nc.sbuf_tensor("u_sb", [F2, N], f16))
        scratch = ctx.enter_context(nc.sbuf_tensor("scratch", [F2, N], f32))
        wrm = ctx.enter_context(nc.sbuf_tensor("wrm", [P, 512], f16))
        dum = ctx.enter_context(nc.sbuf_tensor("dum", [1, 8], f32))
        acc = ctx.enter_context(nc.sbuf_tensor("acc", [F2, NACC], f32))
        ps = ctx.enter_context(nc.psum_tensor("ps", [P, N], f32))
        gsems = [
            ctx.enter_context(nc.semaphore(f"gsem{g}")) for g in range(len(GROUPS))
        ]
        usem = ctx.enter_context(nc.semaphore("usem"))
        asem = ctx.enter_context(nc.semaphore("asem"))
        wsem = ctx.enter_context(nc.semaphore("wsem"))
        msem = ctx.enter_context(nc.semaphore("msem"))
        pe_sem = ctx.enter_context(nc.semaphore("pe"))
        dve_sem = ctx.enter_context(nc.semaphore("dve"))
        osem = ctx.enter_context(nc.semaphore("osem"))
        block = ctx.enter_context(nc.Block())

        def group_dma(eng, g):
            ks = GROUPS[g]
            eng.dma_start(
                out=big[0:P, OFF[ks[0]]:OFF[ks[-1] + 1]],
                in_=movs[g][0:P, 0:GW[g]],
            ).then_inc(gsems[g], 16)

        @block.sync
        def _(sync):
            for g in range(len(GROUPS) - 1):
                group_dma(sync, g)
            sync.wait_ge(dve_sem, 3)
            sync.dma_start(out=out[:, 0:3], in_=acc[:, 0:3]).then_inc(osem, 16)
            sync.wait_ge(dve_sem, NACC)
            sync.dma_start(out=out[:, 3:NACC], in_=acc[:, 3:NACC]).then_inc(osem, 16)
            sync.wait_ge(osem, 32)

        @block.scalar
        def _(scalar):
            # second HWDGE ring: U (half-height) + the last mask group; the
            # second half of U is duplicated on-chip (SBUF->SBUF, no HBM).
            scalar.dma_start(out=u_sb[0:F, :], in_=u[:, :]).then_inc(usem, 16)
            group_dma(scalar, len(GROUPS) - 1)
            scalar.wait_ge(usem, 16)
            scalar.dma_start(out=u_sb[F:F2, :], in_=u_sb[0:F, :]).then_inc(usem, 16)
            # dummy Copy activation: pulls the ~2.7us ACT table load into idle
            # time, long before the first real reduce needs it
            scalar.wait_ge(asem, 1)
            scalar.activation(
                out=dum[0:1, 0:8], in_=dum[0:1, 0:8],
                func=mybir.ActivationFunctionType.Copy,
            )
            # reduce stage: accumulate each TT'd segment into its acc column
            for i, (c0, c1, thr, col) in enumerate(DVE_ITEMS):
                scalar.wait_ge(msem, i + 1)
                scalar.activation(
                    out=scratch[0:F2, c0:c1], in_=scratch[0:F2, c0:c1],
                    func=mybir.ActivationFunctionType.Copy,
                    accum_out=acc[0:F2, col:col + 1],
                ).then_inc(dve_sem, 1)

        @block.tensor
        def _(tensor):
            # HAM warm-up: junk f16 matmuls into rows that chunk 0 later
            # overwrites with start=True; busies the PE during group 0's DMA
            # so the 2.4 GHz un-throttle lands before the real stream.
            tensor.wait_ge(wsem, 1)
            for w in range(8):
                tensor.matmul(
                    ps[0:32, 0:512],
                    lhsT=wrm[0:P, 0:32],
                    rhs=wrm[0:P, 0:512],
                    start=True,
                    stop=True,
                )
            for k in range(NCHUNK):
                if k == GROUPS[group_of[k]][0]:
                    tensor.wait_ge(gsems[group_of[k]], 16)
                lhsT = big[0:P, OFF[k]:OFF[k] + F2]
                wbase = OFF[k] + F2
                c0 = P * k
                mm = None
                while c0 < N:
                    c1 = min(N, (c0 // 512 + 1) * 512)
                    bank = c0 // 512
                    mm = tensor.matmul(
                        ps[0:F2, c0:c1],
                        lhsT=lhsT,
                        rhs=big[0:P, wbase + (c0 - P * k):wbase + (c1 - P * k)],
                        start=(k == 0),
                        stop=(k == 4 * bank + 3),
                    )
                    c0 = c1
                mm.then_inc(pe_sem, 1)

        @block.vector
        def _(vector):
            vector.memset(wrm[0:P, 0:512], 0.0).then_inc(wsem, 1)
            vector.memset(dum[0:1, 0:8], 0.0).then_inc(asem, 1)
            for i, (c0, c1, thr, col) in enumerate(DVE_ITEMS):
                vector.wait_ge(pe_sem, thr)
                if i == 0:
                    vector.wait_ge(usem, 32)  # U + duplicate resident
                vector.tensor_mul(
                    scratch[0:F2, c0:c1],
                    ps[0:F2, c0:c1],
                    u_sb[0:F2, c0:c1],
                ).then_inc(msem, 1)

    nc.compile()
    return nc


_TRIU128 = None


def _features(pos):
    """pos [B, N, 2] float64 -> V [B, N, F] float64."""
    x, y = pos[:, :, 0], pos[:, :, 1]
    r = x * x + y * y
    damp = np.exp(-r / TEMPERATURE)
    xp = [np.ones_like(x)]
    yp = [np.ones_like(y)]
    for _ in range(KDEG):
        xp.append(xp[-1] * x)
        yp.append(yp[-1] * y)
    cols = []
    for k in range(KDEG + 1):
        for t in range(k + 1):
            c = np.sqrt((2.0 / TEMPERATURE) ** k * comb(k, t) / factorial(k))
            cols.append(damp * c * xp[t] * yp[k - t])
    return np.stack(cols, axis=2)


def _host_prep(embedding, abs_coords, patch_mask):
    global _TRIU128
    if _TRIU128 is None:
        _TRIU128 = np.triu(np.ones((P, P), dtype=np.uint8), k=1)

    pos = embedding.astype(np.float64) + abs_coords.astype(np.float64)
    V = _features(pos)                                   # [B, N, F] f64
    Vh = V.astype(FP8)
    Vl = (V - Vh.astype(np.float64)).astype(FP8)
    Vh_u8 = Vh.view(np.uint8)
    Vl_u8 = Vl.view(np.uint8)
    U = V.astype(np.float16)                             # [B, N, F]

    lut = np.array([0.0, 1.0, 2.0], dtype=FP8).view(np.uint8)  # msum -> fp8 byte

    in_maps = []
    for b in range(B):
        mb = (patch_mask[b] == 1).astype(np.uint8)
        im = {}
        for g, ks in enumerate(GROUPS):
            mg = np.empty((P, GW[g]), dtype=np.uint8)
            o = 0
            for k in ks:
                rs = slice(P * k, P * (k + 1))
                mg[:, o:o + F] = Vh_u8[b, rs]
                mg[:, o + F:o + F2] = Vl_u8[b, rs]
                msum = mb[rs, P * k:] + mb[P * k:, rs].T   # [128, W_k] in {0,1,2}
                msum[:, :P] *= _TRIU128                    # strict upper, diag block
                mg[:, o + F2:o + CHUNKW[k]] = lut[msum]
                o += CHUNKW[k]
            im[f"mov{g}"] = mg.view(FP8)
        im["u"] = np.ascontiguousarray(U[b].T)
        in_maps.append(im)
    return in_maps


def kernel(embedding, abs_coords, patch_mask):
    global LAST_RESULTS
    from concourse.bass_utils import run_bass_kernel_spmd

    embedding = np.asarray(embedding)
    abs_coords = np.asarray(abs_coords)
    patch_mask = np.asarray(patch_mask)

    if "nc" not in _cache:
        _cache["nc"] = _build()
    nc = _cache["nc"]

    in_maps = _host_prep(embedding, abs_coords, patch_mask)

    res = run_bass_kernel_spmd(
        nc, in_maps, core_ids=list(range(B)),
        trace=TRACE, trace_cores=[0] if TRACE else None,
    )
    LAST_RESULTS = res

    t_hw = sum(res.results[b]["out"].astype(np.float64).sum() for b in range(B))
    count = np.count_nonzero(patch_mask == 1)
    diag_cnt = sum(
        int(np.trace((patch_mask[b] == 1).astype(np.int64))) for b in range(B)
    )
    loss = np.float64(count) - np.float64(diag_cnt) - t_hw
    return np.array(loss, dtype=np.float32)
